# revision 14
# baseline (speedup 1.0000x reference)
"""Trainium2 Bass kernel for the CNN-VAE loss:

    prob = einsum('klb,hwb->klhw', beta, A) * 5000
    mse  = mean(sum(|x - prob[:, :, None]|^2, axis=1))

Strategy
--------
K*L = 128 == SBUF partition count, so (k,l) lives on partitions and the
40000-pixel hw axis is sharded across the 8 cores (5000 pixels each).

x, A, beta are cast to bf16 on the host (halves the 61MB HBM stream;
the loss is dominated by prob^2 ~ (5000*beta.A)^2 so bf16 error is far
below the 2e-2 tolerance). beta^T arrives pre-scaled by 5000 as a tiny
separate tensor so the PE can start before the A^T stream lands.

Per core, pipelined over 5 groups of 1000 pixels:
  PE:   prob group = bts^T.T @ A^T chunks into PSUM f32 (2 banks)
  cast: prob PSUM f32 -> SBUF bf16 (ACT Copy or DVE, balanced)
  sub:  x -= prob  (DVE tensor_tensor 2x-mode bf16, prob broadcast
        over c; one group on GPSIMD to offload the DVE)
  square+accumulate into a per-group accumulator column: ACT
  (activation Square, accum_out) or DVE (scalar_tensor_tensor
  bypass/mult with accum_out), split to balance engine load.
A dummy Square at kernel start pulls the ACT table load (~2.7us)
under the DMA stream head.

The (128,5) accumulator is DMA'd out; the host sums partials across
columns, partitions, and cores, and divides by 16*3*200*200.
"""

import numpy as np

K, L, NB, H, W = 16, 8, 3, 200, 200
KL = K * L          # 128 partitions
C = 3               # broadcast channel dim of x
HW = H * W          # 40000
N_CORES = 8
HW_SHARD = HW // N_CORES   # 5000
MCHUNK = 500               # matmul chunk (one PSUM bank)
GROUP = 1000               # pixels per steady-state iteration
N_GROUPS = HW_SHARD // GROUP    # 5
SCALE = 5000.0
DENOM = float(K * C * H * W)  # mean over [K, C, H, W] after summing L

SQ_DVE = (4,)     # groups whose square+reduce runs fully on DVE
SQ_SPLIT_G = 3    # this group's square is split DVE/ACT at SQ_SPLIT_PX
SQ_SPLIT_PX = 500
CP_DVE = ()       # groups whose PSUM->SBUF prob cast runs on DVE

_NC = None


def _build():
    global _NC
    if _NC is not None:
        return _NC
    from contextlib import ExitStack

    import concourse.bacc as bacc
    import concourse.mybir as mybir
    import concourse.tile as tile

    f32 = mybir.dt.float32
    bf16 = mybir.dt.bfloat16
    nc = bacc.Bacc("TRN2", target_bir_lowering=False, debug=False)

    xs = nc.dram_tensor("xs", [KL, C, HW_SHARD], bf16, kind="ExternalInput").ap()
    bt = nc.dram_tensor("bt", [NB, KL], bf16, kind="ExternalInput").ap()
    at = nc.dram_tensor("at", [NB, HW_SHARD], bf16, kind="ExternalInput").ap()
    out = nc.dram_tensor("out", [KL, N_GROUPS + 1], f32, kind="ExternalOutput").ap()

    with tile.TileContext(nc) as tc, ExitStack() as ctx:
        const = ctx.enter_context(tc.tile_pool(name="const", bufs=1))
        xpool = ctx.enter_context(tc.tile_pool(name="x", bufs=N_GROUPS))
        ppool = ctx.enter_context(tc.tile_pool(name="psum", bufs=3, space="PSUM"))
        pbpool = ctx.enter_context(tc.tile_pool(name="pb", bufs=3))

        bts = const.tile([NB, KL], bf16)
        nc.sync.dma_start(bts[:], bt[:])
        at_sb = const.tile([NB, HW_SHARD], bf16)
        nc.sync.dma_start(at_sb[:], at[:])

        # Pull the ACT Square table load in before the real work.
        warm = const.tile([1, 2], f32)
        nc.vector.memset(warm[:], 0.0)
        nc.scalar.activation(warm[:], warm[:], mybir.ActivationFunctionType.Square)

        xts = []
        for g in range(N_GROUPS):
            xt = xpool.tile([KL, C, GROUP], bf16)
            nc.sync.dma_start(xt[:], xs[:, :, g * GROUP : (g + 1) * GROUP])
            xts.append(xt)

        acc = const.tile([KL, N_GROUPS + 1], f32)

        BANK = 512  # PSUM bank width in f32; matmul output must stay in-bank
        for g in range(N_GROUPS):
            pp = ppool.tile([KL, 2 * BANK], f32)  # two PSUM banks
            for h in range(GROUP // MCHUNK):
                ci = g * (GROUP // MCHUNK) + h
                nc.tensor.matmul(
                    pp[:, h * BANK : h * BANK + MCHUNK],
                    bts[:],
                    at_sb[:, ci * MCHUNK : (ci + 1) * MCHUNK],
                    start=True,
                    stop=True,
                )
            pv = pp[:].rearrange("p (u f) -> p u f", f=BANK)[:, :, :MCHUNK]
            pb = pbpool.tile([KL, GROUP], bf16)
            pbv = pb[:].rearrange("p (u f) -> p u f", f=MCHUNK)
            if g in CP_DVE:
                nc.vector.tensor_copy(pbv, pv)  # PSUM f32 -> SBUF bf16
            else:
                nc.scalar.copy(pbv, pv)

            xt = xts[g]
            xv = xt[:]
            pb_b = pb[:].unsqueeze(1).broadcast_to([KL, C, GROUP])
            nc.vector.tensor_sub(xv, xv, pb_b)
            if g == SQ_SPLIT_G:
                nc.scalar.activation(
                    xt[:, :, SQ_SPLIT_PX:],
                    xt[:, :, SQ_SPLIT_PX:],
                    mybir.ActivationFunctionType.Square,
                    accum_out=acc[:, g : g + 1],
                )
            elif g not in SQ_DVE:
                nc.scalar.activation(
                    xv,
                    xv,
                    mybir.ActivationFunctionType.Square,
                    accum_out=acc[:, g : g + 1],
                )

        # DVE square share: late groups, so the subtract chain (which
        # gates the ACT squares) is never blocked behind them.
        xsp = xts[SQ_SPLIT_G][:, :, :SQ_SPLIT_PX]
        nc.vector.scalar_tensor_tensor(
            out=xsp,
            in0=xsp,
            scalar=0.0,
            in1=xsp,
            op0=mybir.AluOpType.bypass,
            op1=mybir.AluOpType.mult,
            accum_out=acc[:, N_GROUPS : N_GROUPS + 1],
        )
        for g in SQ_DVE:
            xv = xts[g][:]
            nc.vector.scalar_tensor_tensor(
                out=xv,
                in0=xv,
                scalar=0.0,
                in1=xv,
                op0=mybir.AluOpType.bypass,
                op1=mybir.AluOpType.mult,
                accum_out=acc[:, g : g + 1],
            )

        nc.sync.dma_start(out[:], acc[:])

    nc.compile()
    _NC = nc
    return nc


def _make_in_maps(x, beta, A):
    import ml_dtypes

    x = np.asarray(x, dtype=np.float32)
    beta = np.ascontiguousarray(np.asarray(beta, dtype=np.float32))
    A = np.ascontiguousarray(np.asarray(A, dtype=np.float32))

    xb = x.reshape(KL, C, HW).astype(ml_dtypes.bfloat16)
    at_full = A.reshape(HW, NB).T                      # (3, 40000)
    btm = np.ascontiguousarray(
        (beta.reshape(KL, NB).T * SCALE).astype(ml_dtypes.bfloat16)
    )  # (3, 128), pre-scaled

    in_maps = []
    for i in range(N_CORES):
        sl = slice(i * HW_SHARD, (i + 1) * HW_SHARD)
        in_maps.append(
            {
                "xs": np.ascontiguousarray(xb[:, :, sl]),
                "bt": btm,
                "at": np.ascontiguousarray(at_full[:, sl].astype(ml_dtypes.bfloat16)),
            }
        )
    return in_maps


def _run(in_maps, trace=False, **kwargs):
    from concourse import bass_utils

    nc = _build()
    return bass_utils.run_bass_kernel_spmd(
        nc, in_maps, list(range(N_CORES)), trace=trace, **kwargs
    )


def _combine(results):
    total = 0.0
    for r in results:
        total += float(np.sum(np.asarray(r["out"], dtype=np.float64)))
    return np.float32(total / DENOM)


def kernel(x, beta, A):
    res = _run(_make_in_maps(x, beta, A))
    return _combine(res.results)


# revision 16
# speedup vs baseline: 1.0132x; 1.0132x over previous
"""Trainium2 Bass kernel for the CNN-VAE loss:

    prob = einsum('klb,hwb->klhw', beta, A) * 5000
    mse  = mean(sum(|x - prob[:, :, None]|^2, axis=1))

Strategy
--------
K*L = 128 == SBUF partition count, so (k,l) lives on partitions and the
40000-pixel hw axis is sharded across the 8 cores (5000 pixels each).

x, A, beta are cast to bf16 on the host (halves the 61MB HBM stream;
the loss is dominated by prob^2 ~ (5000*beta.A)^2 so bf16 error is far
below the 2e-2 tolerance). beta^T arrives pre-scaled by 5000 as a tiny
separate tensor so the PE can start before the A^T stream lands.

Per core, pipelined over 5 groups of 1000 pixels:
  PE:   prob group = bts^T.T @ A^T chunks into PSUM f32 (2 banks)
  cast: prob PSUM f32 -> SBUF bf16 (ACT Copy or DVE, balanced)
  sub:  x -= prob  (DVE tensor_tensor 2x-mode bf16, prob broadcast
        over c; one group on GPSIMD to offload the DVE)
  square+accumulate into a per-group accumulator column: ACT
  (activation Square, accum_out) or DVE (scalar_tensor_tensor
  bypass/mult with accum_out), split to balance engine load.
A dummy Square at kernel start pulls the ACT table load (~2.7us)
under the DMA stream head.

The (128,5) accumulator is DMA'd out; the host sums partials across
columns, partitions, and cores, and divides by 16*3*200*200.
"""

import numpy as np

K, L, NB, H, W = 16, 8, 3, 200, 200
KL = K * L          # 128 partitions
C = 3               # broadcast channel dim of x
HW = H * W          # 40000
N_CORES = 8
HW_SHARD = HW // N_CORES   # 5000
MCHUNK = 500               # matmul chunk (one PSUM bank)
GROUP = 1000               # pixels per steady-state iteration
N_GROUPS = HW_SHARD // GROUP    # 5
SCALE = 5000.0
DENOM = float(K * C * H * W)  # mean over [K, C, H, W] after summing L

SQ_DVE = (4,)     # groups whose square+reduce runs fully on DVE
SQ_SPLIT_G = 3    # this group's square is split DVE/ACT at SQ_SPLIT_PX
SQ_SPLIT_PX = 500
CP_DVE = ()       # groups whose PSUM->SBUF prob cast runs on DVE

_NC = None


def _build():
    global _NC
    if _NC is not None:
        return _NC
    from contextlib import ExitStack

    import concourse.bacc as bacc
    import concourse.mybir as mybir
    import concourse.tile as tile

    f32 = mybir.dt.float32
    bf16 = mybir.dt.bfloat16
    nc = bacc.Bacc("TRN2", target_bir_lowering=False, debug=False)

    xs = nc.dram_tensor("xs", [KL, C, HW_SHARD], bf16, kind="ExternalInput").ap()
    bt = nc.dram_tensor("bt", [NB, KL], bf16, kind="ExternalInput").ap()
    at = nc.dram_tensor("at", [NB, HW_SHARD], bf16, kind="ExternalInput").ap()
    out = nc.dram_tensor("out", [KL, N_GROUPS + 1], f32, kind="ExternalOutput").ap()

    with tile.TileContext(nc) as tc, ExitStack() as ctx:
        const = ctx.enter_context(tc.tile_pool(name="const", bufs=1))
        xpool = ctx.enter_context(tc.tile_pool(name="x", bufs=N_GROUPS))
        ppool = ctx.enter_context(tc.tile_pool(name="psum", bufs=3, space="PSUM"))
        pbpool = ctx.enter_context(tc.tile_pool(name="pb", bufs=3))

        bts = const.tile([NB, KL], bf16)
        nc.sync.dma_start(bts[:], bt[:])
        at_sb = const.tile([NB, HW_SHARD], bf16)
        nc.sync.dma_start(at_sb[:], at[:])

        # Pull the ACT Square table load in before the real work.
        warm = const.tile([1, 2], f32)
        nc.vector.memset(warm[:], 0.0)
        nc.scalar.activation(warm[:], warm[:], mybir.ActivationFunctionType.Square)

        # Zero bias fed to every ACT Square; its writer is emitted after
        # the last prob cast so the in-order ACT engine finishes all casts
        # (which gate the DVE subtract chain) before starting squares.
        z128 = const.tile([KL, 1], f32)
        nc.vector.memset(z128[:], 0.0)
        bias0 = const.tile([KL, 1], f32)

        xts = []
        for g in range(N_GROUPS):
            xt = xpool.tile([KL, C, GROUP], bf16)
            nc.sync.dma_start(xt[:], xs[:, :, g * GROUP : (g + 1) * GROUP])
            xts.append(xt)

        acc = const.tile([KL, N_GROUPS + 1], f32)

        BANK = 512  # PSUM bank width in f32; matmul output must stay in-bank
        for g in range(N_GROUPS):
            pp = ppool.tile([KL, 2 * BANK], f32)  # two PSUM banks
            for h in range(GROUP // MCHUNK):
                ci = g * (GROUP // MCHUNK) + h
                nc.tensor.matmul(
                    pp[:, h * BANK : h * BANK + MCHUNK],
                    bts[:],
                    at_sb[:, ci * MCHUNK : (ci + 1) * MCHUNK],
                    start=True,
                    stop=True,
                )
            pv = pp[:].rearrange("p (u f) -> p u f", f=BANK)[:, :, :MCHUNK]
            pb = pbpool.tile([KL, GROUP], bf16)
            pbv = pb[:].rearrange("p (u f) -> p u f", f=MCHUNK)
            if g in CP_DVE:
                nc.vector.tensor_copy(pbv, pv)  # PSUM f32 -> SBUF bf16
            else:
                nc.scalar.copy(pbv, pv)
            if g == N_GROUPS - 1:
                nc.scalar.copy(bias0[:], z128[:])

            xt = xts[g]
            xv = xt[:]
            pb_b = pb[:].unsqueeze(1).broadcast_to([KL, C, GROUP])
            nc.vector.tensor_sub(xv, xv, pb_b)
            if g == SQ_SPLIT_G:
                nc.scalar.activation(
                    xt[:, :, SQ_SPLIT_PX:],
                    xt[:, :, SQ_SPLIT_PX:],
                    mybir.ActivationFunctionType.Square,
                    bias=bias0[:],
                    accum_out=acc[:, g : g + 1],
                )
            elif g not in SQ_DVE:
                nc.scalar.activation(
                    xv,
                    xv,
                    mybir.ActivationFunctionType.Square,
                    bias=bias0[:],
                    accum_out=acc[:, g : g + 1],
                )

        # DVE square share: late groups, so the subtract chain (which
        # gates the ACT squares) is never blocked behind them.
        xsp = xts[SQ_SPLIT_G][:, :, :SQ_SPLIT_PX]
        nc.vector.scalar_tensor_tensor(
            out=xsp,
            in0=xsp,
            scalar=0.0,
            in1=xsp,
            op0=mybir.AluOpType.bypass,
            op1=mybir.AluOpType.mult,
            accum_out=acc[:, N_GROUPS : N_GROUPS + 1],
        )
        for g in SQ_DVE:
            xv = xts[g][:]
            nc.vector.scalar_tensor_tensor(
                out=xv,
                in0=xv,
                scalar=0.0,
                in1=xv,
                op0=mybir.AluOpType.bypass,
                op1=mybir.AluOpType.mult,
                accum_out=acc[:, g : g + 1],
            )

        nc.sync.dma_start(out[:], acc[:])

    nc.compile()
    _NC = nc
    return nc


def _make_in_maps(x, beta, A):
    import ml_dtypes

    x = np.asarray(x, dtype=np.float32)
    beta = np.ascontiguousarray(np.asarray(beta, dtype=np.float32))
    A = np.ascontiguousarray(np.asarray(A, dtype=np.float32))

    xb = x.reshape(KL, C, HW).astype(ml_dtypes.bfloat16)
    at_full = A.reshape(HW, NB).T                      # (3, 40000)
    btm = np.ascontiguousarray(
        (beta.reshape(KL, NB).T * SCALE).astype(ml_dtypes.bfloat16)
    )  # (3, 128), pre-scaled

    in_maps = []
    for i in range(N_CORES):
        sl = slice(i * HW_SHARD, (i + 1) * HW_SHARD)
        in_maps.append(
            {
                "xs": np.ascontiguousarray(xb[:, :, sl]),
                "bt": btm,
                "at": np.ascontiguousarray(at_full[:, sl].astype(ml_dtypes.bfloat16)),
            }
        )
    return in_maps


def _run(in_maps, trace=False, **kwargs):
    from concourse import bass_utils

    nc = _build()
    return bass_utils.run_bass_kernel_spmd(
        nc, in_maps, list(range(N_CORES)), trace=trace, **kwargs
    )


def _combine(results):
    total = 0.0
    for r in results:
        total += float(np.sum(np.asarray(r["out"], dtype=np.float64)))
    return np.float32(total / DENOM)


def kernel(x, beta, A):
    res = _run(_make_in_maps(x, beta, A))
    return _combine(res.results)


# revision 17
# speedup vs baseline: 1.1907x; 1.1752x over previous
"""Trainium2 Bass kernel for the CNN-VAE loss:

    prob = einsum('klb,hwb->klhw', beta, A) * 5000
    mse  = mean(sum(|x - prob[:, :, None]|^2, axis=1))

Strategy
--------
K*L = 128 == SBUF partition count, so (k,l) lives on partitions and the
40000-pixel hw axis is sharded across the 8 cores (5000 pixels each).

x, A, beta are cast to bf16 on the host (halves the 61MB HBM stream;
the loss is dominated by prob^2 ~ (5000*beta.A)^2 so bf16 error is far
below the 2e-2 tolerance). beta^T arrives pre-scaled by 5000 as a tiny
separate tensor so the PE can start before the A^T stream lands.

Per core, pipelined over 5 groups of 1000 pixels:
  PE:   prob group = bts^T.T @ A^T chunks into PSUM f32 (2 banks)
  cast: prob PSUM f32 -> SBUF bf16 (ACT Copy or DVE, balanced)
  sub:  x -= prob  (DVE tensor_tensor 2x-mode bf16, prob broadcast
        over c; one group on GPSIMD to offload the DVE)
  square+accumulate into a per-group accumulator column: ACT
  (activation Square, accum_out) or DVE (scalar_tensor_tensor
  bypass/mult with accum_out), split to balance engine load.
A dummy Square at kernel start pulls the ACT table load (~2.7us)
under the DMA stream head.

The (128,5) accumulator is DMA'd out; the host sums partials across
columns, partitions, and cores, and divides by 16*3*200*200.
"""

import numpy as np

K, L, NB, H, W = 16, 8, 3, 200, 200
KL = K * L          # 128 partitions
C = 3               # broadcast channel dim of x
HW = H * W          # 40000
N_CORES = 8
HW_SHARD = HW // N_CORES   # 5000
MCHUNK = 500               # matmul chunk (one PSUM bank)
GROUP = 1000               # pixels per steady-state iteration
N_GROUPS = HW_SHARD // GROUP    # 5
SCALE = 5000.0
DENOM = float(K * C * H * W)  # mean over [K, C, H, W] after summing L

SQ_DVE = (4,)     # groups whose square+reduce runs fully on DVE
SQ_SPLIT_G = 3    # this group's square is split DVE/ACT at SQ_SPLIT_PX
SQ_SPLIT_PX = 500
CP_DVE = ()       # groups whose PSUM->SBUF prob cast runs on DVE

_NC = None


def _build():
    global _NC
    if _NC is not None:
        return _NC
    from contextlib import ExitStack

    import concourse.bacc as bacc
    import concourse.mybir as mybir
    import concourse.tile as tile

    f32 = mybir.dt.float32
    bf16 = mybir.dt.bfloat16
    nc = bacc.Bacc("TRN2", target_bir_lowering=False, debug=False)

    xs = nc.dram_tensor("xs", [KL, C, HW_SHARD], bf16, kind="ExternalInput").ap()
    bt = nc.dram_tensor("bt", [NB, KL], bf16, kind="ExternalInput").ap()
    at = nc.dram_tensor("at", [NB, HW_SHARD], bf16, kind="ExternalInput").ap()
    out = nc.dram_tensor("out", [KL, N_GROUPS + 1], f32, kind="ExternalOutput").ap()

    with tile.TileContext(nc) as tc, ExitStack() as ctx:
        const = ctx.enter_context(tc.tile_pool(name="const", bufs=1))
        xpool = ctx.enter_context(tc.tile_pool(name="x", bufs=N_GROUPS))
        ppool = ctx.enter_context(tc.tile_pool(name="psum", bufs=3, space="PSUM"))
        pbpool = ctx.enter_context(tc.tile_pool(name="pb", bufs=4))

        bts = const.tile([NB, KL], bf16)
        nc.sync.dma_start(bts[:], bt[:])
        at_sb = const.tile([NB, HW_SHARD], bf16)
        nc.sync.dma_start(at_sb[:], at[:])

        # Pull the ACT Square table load in before the real work.
        warm = const.tile([1, 2], f32)
        nc.vector.memset(warm[:], 0.0)
        nc.scalar.activation(warm[:], warm[:], mybir.ActivationFunctionType.Square)

        xts = []
        for g in range(N_GROUPS):
            xt = xpool.tile([KL, C, GROUP], bf16)
            nc.sync.dma_start(xt[:], xs[:, :, g * GROUP : (g + 1) * GROUP])
            xts.append(xt)

        acc = const.tile([KL, N_GROUPS + 1], f32)

        BANK = 512  # PSUM bank width in f32; matmul output must stay in-bank
        # Emission order = scheduler priority: matmuls+casts first, then
        # subs, then squares, so ready casts always beat ready squares on
        # ACT and the subtract chain is never starved.
        pbs = []
        for g in range(N_GROUPS):
            pp = ppool.tile([KL, 2 * BANK], f32)  # two PSUM banks
            for h in range(GROUP // MCHUNK):
                ci = g * (GROUP // MCHUNK) + h
                nc.tensor.matmul(
                    pp[:, h * BANK : h * BANK + MCHUNK],
                    bts[:],
                    at_sb[:, ci * MCHUNK : (ci + 1) * MCHUNK],
                    start=True,
                    stop=True,
                )
            pv = pp[:].rearrange("p (u f) -> p u f", f=BANK)[:, :, :MCHUNK]
            pb = pbpool.tile([KL, GROUP], bf16)
            pbv = pb[:].rearrange("p (u f) -> p u f", f=MCHUNK)
            if g in CP_DVE:
                nc.vector.tensor_copy(pbv, pv)  # PSUM f32 -> SBUF bf16
            else:
                nc.scalar.copy(pbv, pv)
            pbs.append(pb)

        for g in range(N_GROUPS):
            xv = xts[g][:]
            pb_b = pbs[g][:].unsqueeze(1).broadcast_to([KL, C, GROUP])
            nc.vector.tensor_sub(xv, xv, pb_b)

        for g in range(N_GROUPS):
            xt = xts[g]
            if g == SQ_SPLIT_G:
                nc.scalar.activation(
                    xt[:, :, SQ_SPLIT_PX:],
                    xt[:, :, SQ_SPLIT_PX:],
                    mybir.ActivationFunctionType.Square,
                    accum_out=acc[:, g : g + 1],
                )
            elif g not in SQ_DVE:
                nc.scalar.activation(
                    xt[:],
                    xt[:],
                    mybir.ActivationFunctionType.Square,
                    accum_out=acc[:, g : g + 1],
                )

        xsp = xts[SQ_SPLIT_G][:, :, :SQ_SPLIT_PX]
        nc.vector.scalar_tensor_tensor(
            out=xsp,
            in0=xsp,
            scalar=0.0,
            in1=xsp,
            op0=mybir.AluOpType.bypass,
            op1=mybir.AluOpType.mult,
            accum_out=acc[:, N_GROUPS : N_GROUPS + 1],
        )
        for g in SQ_DVE:
            xv = xts[g][:]
            nc.vector.scalar_tensor_tensor(
                out=xv,
                in0=xv,
                scalar=0.0,
                in1=xv,
                op0=mybir.AluOpType.bypass,
                op1=mybir.AluOpType.mult,
                accum_out=acc[:, g : g + 1],
            )

        nc.sync.dma_start(out[:], acc[:])

    nc.compile()
    _NC = nc
    return nc


def _make_in_maps(x, beta, A):
    import ml_dtypes

    x = np.asarray(x, dtype=np.float32)
    beta = np.ascontiguousarray(np.asarray(beta, dtype=np.float32))
    A = np.ascontiguousarray(np.asarray(A, dtype=np.float32))

    xb = x.reshape(KL, C, HW).astype(ml_dtypes.bfloat16)
    at_full = A.reshape(HW, NB).T                      # (3, 40000)
    btm = np.ascontiguousarray(
        (beta.reshape(KL, NB).T * SCALE).astype(ml_dtypes.bfloat16)
    )  # (3, 128), pre-scaled

    in_maps = []
    for i in range(N_CORES):
        sl = slice(i * HW_SHARD, (i + 1) * HW_SHARD)
        in_maps.append(
            {
                "xs": np.ascontiguousarray(xb[:, :, sl]),
                "bt": btm,
                "at": np.ascontiguousarray(at_full[:, sl].astype(ml_dtypes.bfloat16)),
            }
        )
    return in_maps


def _run(in_maps, trace=False, **kwargs):
    from concourse import bass_utils

    nc = _build()
    return bass_utils.run_bass_kernel_spmd(
        nc, in_maps, list(range(N_CORES)), trace=trace, **kwargs
    )


def _combine(results):
    total = 0.0
    for r in results:
        total += float(np.sum(np.asarray(r["out"], dtype=np.float64)))
    return np.float32(total / DENOM)


def kernel(x, beta, A):
    res = _run(_make_in_maps(x, beta, A))
    return _combine(res.results)
